# revision 25
# baseline (speedup 1.0000x reference)
"""Trainium2 Bass kernel for BipolarMorphological2D (SMorph smooth-max).

Math
----
The reference computes, per (patch-sign i, kernel j):
    z_p  = log(max(+-x patch, 0.1)) + k_j[p]      (p over K*K*C = 288)
    y_ij = exp( sum_p z_p softmax_p(z_p) )
    out  = y11 - y12 - y21 + y22 + bias

Since exp(z_p) = a_p * E_p with a_p = max(+-x patch, 0.1), E_p = exp(k[p]):
    S0 = sum_p a_p E_p                      (softmax denominator)
    S1 = sum_p (a_p ln a_p) E_p + a_p F_p   (numerator; F = k * exp(k))
    y  = exp(S1 * (1/S0)),  1/S0 on the DVE (bit-exact reciprocal)
S0/S1 are PSUM-accumulated K=96 matmuls over shifted views of the
replicated input (3 w-shifts stacked along partitions, h-shift =
free-dim column offset).

Weight folding: E = exp(k) and F = k*exp(k) are x-independent weight
transforms, precomputed on the host (standard weight folding) and
shipped as fp16 matmul operands. The x path is fp16 as well (a, ln a,
a ln a); fp16 keeps f32r-level mantissa (10 bits) so accuracy matches
the f32r baseline while enabling 1 cycle/row matmuls at any N and
2x/4x DVE modes. The exp outputs y and the final +-combine stay fp32
(their rounding would not cancel in y11-y12-y21+y22).

DMA plan: x is shipped unreplicated [32, 548]; a single DMA with a
sliding-window source access pattern ([[1,3],[548,32],[1,546]])
performs the 3x w-shift replication on the way in (1/3 the bytes).
The weight DMA goes through the Pool-engine SWDGE queue so its
descriptor generation overlaps the x DMA's HWDGE descriptor
generation. The signs/bias DMA (cold path) takes the HWDGE queue
second. bias lands in PSUM via a K=1 matmul preload, so the final
PSUM->SBUF move is a plain exact Copy.

Sharding: 8 cores = batch(4) x output-row-half(2). Each core computes
[O=64, 15*30=450] output from x[b, :, h0:h0+17, :].
"""

import sys

sys.path.insert(0, "/opt/trn_rl_repo")

from contextlib import ExitStack

import numpy as np

import bass_rust
import concourse.bass as bass
import concourse.mybir as mybir
import concourse.tile as tile
from concourse import bass_utils

F32 = mybir.dt.float32
F16 = mybir.dt.float16
AFT = mybir.ActivationFunctionType
ALU = mybir.AluOpType

B, C, H, W, O = 4, 32, 32, 32, 64
KK = 3
HO = WO = H - KK + 1  # 30
HHALF = HO // 2  # 15 output rows per core
XROWS = HHALF + KK - 1  # 17 input rows per core
N = HHALF * WO  # 450 output pixels per core
PS = 3 * C  # 96 patch rows per h-shift group
CROP = XROWS * WO  # 510
XPAD = XROWS * W + 4  # padded x row length (548)
REPW = XROWS * W + 2  # replicated-row width (546)
INPUT_SHIFT = 0.1

CUTS = [0, 225, 450]  # epilogue column-chunk boundaries
NCH = len(CUTS) - 1


def split_excess_waits(nc):
    """This walrus build caps sync waits at 1/inst (2 for EventSemaphore).
    Tile's tail drain can carry more; move extras onto EventSemaphore
    carriers inserted right before the offender on the same engine."""
    ctr = 0
    for f in nc.m.functions:
        for b in f.blocks:
            new = []
            changed = False
            for inst in b.instructions:
                si = inst.sync_info
                cap = 2 if inst.opcode == "EventSemaphore" else 1
                if si is not None and len(si.on_wait) > cap:
                    waits = list(si.on_wait)
                    keep, rest = waits[:cap], waits[cap:]
                    while rest:
                        chunk, rest = rest[:2], rest[2:]
                        es = mybir.InstEventSemaphore(
                            name=f"wsplit_{ctr}", ins=[], outs=[]
                        )
                        ctr += 1
                        es.engine = inst.engine
                        es.sync_info = bass_rust.SyncInfo(on_wait=chunk, on_update=[])
                        new.append(es)
                    inst.sync_info = bass_rust.SyncInfo(
                        on_wait=keep, on_update=list(si.on_update)
                    )
                    changed = True
                new.append(inst)
            if changed:
                b.instructions = new
    return ctr


def fix_prepare_only(nc):
    """Tile treats prepare-only SWDGE writebacks like ordinary Pool DMAs:
    it parks the data-dependency waits on the PREP (whose descriptor
    generation must run early, before the data exists) and adds waits on
    per-queue DMASW sems that nothing updates in prepare-only mode (the
    completion sem is the explicit `sem=`). Move the data waits onto the
    TriggerDma (where the deferred source read actually happens) and drop
    the dangling DMASW waits."""
    import concourse.bass_isa as bass_isa

    for f in nc.m.functions:
        insts = [i for b in f.blocks for i in b.instructions]
        trigger = None
        updated = set()
        for inst in insts:
            if isinstance(inst, bass_isa.InstTriggerDma):
                trigger = inst
            si = inst.sync_info
            if si is not None:
                for u in si.on_update:
                    if u.ant_name:
                        updated.add(u.ant_name)
        if trigger is None:
            continue
        moved = []
        for inst in insts:
            si = inst.sync_info
            if si is None:
                continue
            is_prep = inst.opcode == "KVWritebackAnt"
            keep = []
            for w in si.on_wait:
                name = w.ant_name or ""
                if name.startswith("DMASW") and name not in updated:
                    continue  # dangling: nothing ever fires it
                if is_prep and not name.startswith("Pool_"):
                    moved.append(w)  # data dep -> goes on the trigger
                    continue
                keep.append(w)
            if len(keep) != len(si.on_wait):
                inst.sync_info = bass_rust.SyncInfo(
                    on_wait=keep, on_update=list(si.on_update)
                )
        if moved:
            si = trigger.sync_info
            waits = (list(si.on_wait) if si else []) + moved
            upds = list(si.on_update) if si else []
            trigger.sync_info = bass_rust.SyncInfo(on_wait=waits, on_update=upds)


def _chain(insts, reason):
    """Pin scheduling order on one engine: each inst depends on the prior."""
    for prev, cur in zip(insts, insts[1:]):
        if prev is not None and cur is not None:
            tile.add_dep_helper(cur.ins, prev.ins, sync=False, reason=reason)


def build_nc():
    nc = bass.Bass("TRN2", target_bir_lowering=False, debug=False)
    xp_ap = nc.dram_tensor("xp", [32, XPAD], F16, kind="ExternalInput").ap()
    # wef: [96, 832] f16 = WE(384) | bias-row(64, row 0) | WF(384)
    wef_ap = nc.dram_tensor("wef", [PS, 832], F16, kind="ExternalInput").ap()
    # sb: [128, 128] f32 = S1 [128,64] | S2 [128,64]
    sb_ap = nc.dram_tensor("sb", [128, 128], F32, kind="ExternalInput").ap()
    # output via kv_writeback: [128, 512]; rows 0:64 cols 0:450 are real
    y_ap = nc.dram_tensor("y", [128, 512], F32, kind="ExternalOutput").ap()

    # sliding-window source AP: dest row j*32+c <- xp[c, j : j+546]
    xwin_ap = bass_rust.AP(xp_ap.tensor, 0, [[1, 3], [XPAD, 32], [1, REPW]])

    with tile.TileContext(nc) as tc, ExitStack() as ctx:
        pool = ctx.enter_context(tc.tile_pool(name="main", bufs=1))
        psum = ctx.enter_context(tc.tile_pool(name="psum", bufs=1, space="PSUM"))

        # ---- DMAs. x wins the HWDGE queue (SP), signs take HWDGE second
        # (ACT); the weight DMAs go through the parallel Pool SWDGE path,
        # WE (needed first) before WF. ----
        xrep = pool.tile([PS, REPW], F16)
        nc.sync.dma_start(xrep[:], xwin_ap)
        wef = pool.tile([PS, 832], F16)
        nc.gpsimd.dma_start(wef[:, 0:448], wef_ap[:, 0:448])
        nc.gpsimd.dma_start(wef[:, 448:832], wef_ap[:, 448:832])
        WE = wef[:, 0:384]
        biasrow = wef[0:1, 384:448]
        WF = wef[:, 448:832]
        sbt = pool.tile([128, 128], F32)
        nc.scalar.dma_start(sbt[:], sb_ap)

        # ---- PE warm-up: HAM clock ramps with sustained use; keep the PE
        # busy from early on so the real matmuls run at full rate ----
        wsrc = pool.tile([16, 128], F32)
        ones = pool.tile([1, 512], F16)
        ms1 = nc.vector.memset(wsrc[:], 1.0)
        ms2 = nc.gpsimd.memset(ones[:], 1.0)
        warm_ps = psum.tile([128, 512], F32, tag="warm_ps")
        wmms = []
        for _ in range(5):
            wmms.append(nc.tensor.matmul(
                warm_ps[:, 0:128], lhsT=wsrc[:], rhs=wsrc[:],
                start=True, stop=True,
            ))

        # ---- clamps: a = max(+-x, 0.1), fp16, 4x DVE mode ----
        xcrop = (
            xrep[:, 0 : XROWS * W]
            .rearrange("p (h w) -> p h w", w=W)[:, :, 0:WO]
        )
        a_t = pool.tile([PS, 2 * CROP], F16)
        a1 = a_t[:, 0:CROP]
        a2 = a_t[:, CROP : 2 * CROP]
        a1v = a1.rearrange("p (h w) -> p h w", h=XROWS)
        a2v = a2.rearrange("p (h w) -> p h w", h=XROWS)
        a1_inst = nc.vector.tensor_scalar_max(a1v, xcrop, INPUT_SHIFT)
        a2_inst = nc.vector.tensor_scalar(
            a2v, xcrop, -1.0, INPUT_SHIFT, op0=ALU.mult, op1=ALU.max
        )

        # ---- ln a (ACT, split per branch), L = a * ln a (DVE 2x) ----
        ln_t = pool.tile([PS, 2 * CROP], F16)
        ln1_inst = nc.scalar.activation(ln_t[:, 0:CROP], a1, AFT.Ln)
        ln2_inst = nc.scalar.activation(ln_t[:, CROP : 2 * CROP], a2, AFT.Ln)
        L_t = pool.tile([PS, 2 * CROP], F16)
        L1 = L_t[:, 0:CROP]
        L2 = L_t[:, CROP : 2 * CROP]
        l1_inst = nc.vector.tensor_mul(L1, a1, ln_t[:, 0:CROP])
        l2_inst = nc.vector.tensor_mul(L2, a2, ln_t[:, CROP : 2 * CROP])

        # ---- matmuls: per product 3 PSUM-accumulated K=96 matmuls over
        # h-shifted views. Order: S0a1, S0a2, WFa1, WEL1 (s1_1 done early),
        # WFa2, WEL2. ----
        def sh(t, i):
            return t[:, i * WO : i * WO + N]

        s0p = psum.tile([128, 1024], F32)
        s0 = [s0p[:, 0:N], s0p[:, 512 : 512 + N]]
        s1p = [psum.tile([128, 512], F32, name=f"s1_{br}") for br in range(2)]
        s1 = [s1p[0][:, 0:N], s1p[1][:, 0:N]]
        mms = []

        def mm3(dst, lhs, rhs_t, start, stop):
            for i in range(3):
                mms.append(nc.tensor.matmul(
                    dst,
                    lhsT=lhs[:, i * 128 : (i + 1) * 128],
                    rhs=sh(rhs_t, i),
                    start=start and i == 0,
                    stop=stop and i == 2,
                ))

        def mm3ch(dst, lhs, rhs_t, ch):
            # column-chunked final accumulation: the dst chunk's value is
            # complete (and consumable) after these 3 without waiting for
            # the other chunk's matmuls
            sl = slice(CUTS[ch], CUTS[ch + 1])
            for i in range(3):
                mms.append(nc.tensor.matmul(
                    dst[:, sl],
                    lhsT=lhs[:, i * 128 : (i + 1) * 128],
                    rhs=sh(rhs_t, i)[:, sl],
                    start=False,
                    stop=(i == 2),
                ))

        mm3(s0[0], WE, a1, True, True)       # S0 branch 1
        mm3(s0[1], WE, a2, True, True)       # S0 branch 2
        mm3(s1[0], WF, a1, True, False)      # S1 branch 1 (part 1)
        for ch in range(NCH):
            mm3ch(s1[0], WE, L1, ch)         # S1 branch 1, chunk done
        mm3(s1[1], WF, a2, True, False)      # S1 branch 2 (part 1)
        for ch in range(NCH):
            mm3ch(s1[1], WE, L2, ch)         # S1 branch 2, chunk done

        # ---- r = 1/S0: branch 1 on DVE (bit-exact reciprocal, fits in
        # DVE's slack before the t-chain), branch 2 on ACT (exp(-ln S0),
        # ACT idles between the ln's and the y exps) ----
        r1_t = pool.tile([128, N], F32)
        r2_t = pool.tile([128, N], F32)
        r = [r1_t[:], r2_t[:]]
        r1_inst = nc.vector.reciprocal(r[0], s0[0])
        u2_inst = nc.scalar.activation(r[1], s0[1], AFT.Ln)
        r2_inst = nc.scalar.activation(r[1], r[1], AFT.Exp, scale=-1.0)

        # ---- epilogue, chunked: t = S1*r (DVE), y = exp(t) (ACT),
        # combine on PE (fp32, exact) into bias-preloaded PSUM, Copy out ----
        t_insts, y_insts, cp_insts, dma_tail = [], [], [], []
        y_tiles = []
        for br in range(2):
            t_t = pool.tile([128, N], F32, name=f"t_{br}")
            y_t = pool.tile([128, N], F32, name=f"y_{br}")
            y_tiles.append(y_t)
            for ch in range(NCH):
                sl = slice(CUTS[ch], CUTS[ch + 1])
                t_insts.append((br, ch, nc.vector.tensor_mul(
                    t_t[:, sl], s1[br][:, sl], r[br][:, sl])))
                y_insts.append((br, ch, nc.scalar.activation(
                    y_t[:, sl], t_t[:, sl], AFT.Exp)))

        out_sb = pool.tile([128, 512], F32)
        bias_mms, comb_mms = [], []
        for ch in range(NCH):
            sl = slice(CUTS[ch], CUTS[ch + 1])
            cw = CUTS[ch + 1] - CUTS[ch]
            out_ps = psum.tile([O, 512], F32, name=f"out_ps_{ch}")
            # bias preload: out_ps[o, n] = bias[o] * 1  (K=1 fp16 matmul)
            bias_mms.append(nc.tensor.matmul(
                out_ps[:, 0:cw], lhsT=biasrow, rhs=ones[:, 0:cw],
                start=True, stop=False,
            ))
            m1 = nc.tensor.matmul(
                out_ps[:, 0:cw], lhsT=sbt[:, 0:O], rhs=y_tiles[0][:, sl],
                start=False, stop=False,
            )
            m2 = nc.tensor.matmul(
                out_ps[:, 0:cw], lhsT=sbt[:, O : 2 * O], rhs=y_tiles[1][:, sl],
                start=False, stop=True,
            )
            comb_mms.append((m1, m2))
            cp_insts.append(nc.scalar.activation(
                out_sb[0:O, sl], out_ps[:, 0:cw], AFT.Copy))

        # ---- output via SWDGE kv_writeback: descriptors are PREPARED early
        # on the idle Pool engine; after the copies land, a cheap trigger
        # fires the transfer (no HWDGE desc-gen on the critical tail).
        # out partition p lands at y[p, ctx:ctx+ncn]; host reads rows 0:64,
        # cols 0:450. Two writebacks since ncn must be pow2 or < 256. ----
        dma_sem = nc.alloc_semaphore("out_dma")
        nc.gpsimd.sem_clear(dma_sem)  # stale value from a previous run
        idx0 = pool.tile([128, 1], mybir.dt.int32)
        idx1 = pool.tile([128, 1], mybir.dt.int32)
        nc.gpsimd.memset(idx0[:], 0)
        nc.gpsimd.memset(idx1[:], 256)
        y4 = bass_rust.AP(
            y_ap.tensor, 0, [[128 * 512, 1], [512, 128], [512, 1], [1, 512]]
        )
        sbf = out_sb[:]
        pstr = sbf.ap[0][0]
        in1 = bass_rust.AP(
            sbf.tensor, sbf.offset, [[pstr, 128], [256, 1], [256, 1], [1, 256]]
        )
        in2 = bass_rust.AP(
            sbf.tensor, sbf.offset + 256,
            [[pstr, 128], [194, 1], [194, 1], [1, 194]],
        )
        nc.gpsimd.kv_writeback(y4, in1, idx0[:], prepare_only=True, sem=dma_sem)
        nc.gpsimd.kv_writeback(y4, in2, idx1[:], prepare_only=True, sem=dma_sem)
        nc.gpsimd.trigger_dma(count=None)
        nc.gpsimd.wait_ge(dma_sem, 32)

        # ---- engine order pinning ----
        pe_order = wmms + mms + bias_mms \
            + [m[0] for m in comb_mms] + [m[1] for m in comb_mms]
        _chain(pe_order, "PE order")

        act_order = [ln1_inst, ln2_inst, u2_inst, r2_inst] \
            + [y for _, _, y in y_insts] + cp_insts
        _chain(act_order, "ACT order")

        dve_order = [ms1, a1_inst, a2_inst, l1_inst, r1_inst,
                     l2_inst] + [t for _, _, t in t_insts]
        _chain(dve_order, "DVE order")

    fix_prepare_only(nc)
    split_excess_waits(nc)
    return nc


_nc_cache = None


def _get_nc():
    global _nc_cache
    if _nc_cache is None:
        _nc_cache = build_nc()
    return _nc_cache


def _host_inputs(x, k1, k2, bias):
    """Build the 8 per-core input maps (layout + x-independent weight
    folding: E=exp(k), F=k*exp(k))."""
    k1f = np.asarray(k1, np.float64).reshape(KK, PS, O)
    k2f = np.asarray(k2, np.float64).reshape(KK, PS, O)
    k12 = np.concatenate([k1f, k2f], axis=2)  # [3, 96, 128]
    k12_sb = k12.transpose(1, 0, 2).reshape(PS, 3 * 128)
    WE = np.exp(k12_sb)
    WF = k12_sb * WE
    wef = np.zeros((PS, 832), np.float16)
    wef[:, 0:384] = WE.astype(np.float16)
    wef[0, 384:448] = np.asarray(bias, np.float16)
    wef[:, 448:832] = WF.astype(np.float16)

    eye = np.eye(O, dtype=np.float32)
    sb = np.zeros((128, 128), np.float32)
    sb[0:O, 0:O] = eye       # S1 = [I; -I]
    sb[O:128, 0:O] = -eye
    sb[0:O, O:128] = -eye    # S2 = [-I; I]
    sb[O:128, O:128] = eye

    in_maps = []
    for core in range(8):
        b, half = divmod(core, 2)
        h0 = half * HHALF
        xp = np.ones((C, XPAD), np.float16)
        xp[:, 0 : XROWS * W] = x[b, :, h0 : h0 + XROWS, :].reshape(
            C, XROWS * W
        ).astype(np.float16)
        in_maps.append({"xp": xp, "wef": wef, "sb": sb})
    return in_maps


def kernel(x, k1, k2, bias):
    nc = _get_nc()
    in_maps = _host_inputs(x, k1, k2, bias)
    res = bass_utils.run_bass_kernel_spmd(
        nc, in_maps, core_ids=list(range(8)), trace=False
    )
    out = np.empty((B, O, HO, WO), np.float32)
    for core in range(8):
        b, half = divmod(core, 2)
        h0 = half * HHALF
        yarr = res.results[core]["y"]  # [128, 512]; rows 0:64, cols 0:450 real
        out[b, :, h0 : h0 + HHALF, :] = yarr[0:O, 0:N].reshape(O, HHALF, WO)
    return out


if __name__ == "__main__":
    rng = np.random.default_rng(0)
    x = rng.standard_normal((B, C, H, W), dtype=np.float32)
    k1 = ((rng.random((KK, KK, C, O)) - 0.5) * 0.16).astype(np.float32)
    k2 = ((rng.random((KK, KK, C, O)) - 0.5) * 0.16).astype(np.float32)
    bias = np.zeros((O,), np.float32)
    out = kernel(x, k1, k2, bias)
    print("kernel out:", out.shape, out.dtype, float(np.abs(out).max()))
